# revision 35
# baseline (speedup 1.0000x reference)
"""Trainium2 Bass kernel for nn_DCM_22058952032783 (dynamic-conv CNN).

Strategy: pure data-parallel over batch (B=8 -> 8 NeuronCores, one sample
per core, no collectives).

Per-sample math (reference):
    feats = [x]
    for k in (1, 3, 5):
        pooled = adaptive_avg_pool(y, k)               # [inC, k, k]
        kern   = gk_w @ pooled_mean + gk_b             # [mid, k*k] per-sample dw kernels
        x_in   = tr_w @ x + tr_b                       # [mid, HW]
        dwout  = depthwise(x_in, kern)                 # [mid, HW]
        feats.append(fi_w @ dwout + fi_b)
    out = fo_w @ concat(feats) + fo_b

Host-side exact algebraic folds (weights-only preprocessing):
  - G_i  = fo_blk_i @ fi_w_i   folds the fi convs into fo:
        out = fo_x@x + sum_i G_i @ dwout_i + b'   with b' = fo_b + sum fo_blk_i@fi_b_i
  - The k=1 branch is a per-channel scale:  G1 @ (kern1 * (tr1@x + b1)) =
        [G1 diag(kern1) tr1] @ x + G1@(kern1*b1).  The bracket is built on
    device with one small matmul chain and merged with fo_x -> C1.
Device work per core: 3 remaining dense 1x1 conv stacks on TensorE (tr_k3,
tr_k5, and the fused fo pass), depthwise taps as per-partition-scalar FMAs
on VectorE, PSUM evictions on ScalarE, pooling reductions on VectorE.
"""

import os

import numpy as np

# ---- hardcoded problem shapes (nn_DCM_22058952032783) ----
B, inC, midC, outC, H, W, Hy = 8, 512, 512, 512, 64, 64, 60
HW = H * W            # 4096
P = 128
KC = inC // P         # 4 partition chunks of the channel dims
NBLK = 512            # matmul moving free dim
HALF = 2048           # psum half (4 banks)
PAD = 2
PR = H + 2 * PAD      # 68 padded rows
PCE = W + 2 * PAD     # 68 cols, even-aligned pad buffer
PCO = W + 2           # 66 cols, odd-shifted pad buffer
N_CORES = 8

# column layout of the fused pooled/kern tiles: [k5 (25) | k3 (9) | k1 (1)]
OFF5, OFF3, OFF1 = 0, 25, 34

# depthwise taps routed to TensorE (diag matmuls); the rest go to VectorE.
# Groups are (start, step, count) strided runs of within-branch tap indices so
# each group's diag matrices build with ONE broadcast tensor_tensor op.
PE_GROUPS5 = ((1, 5, 5), (3, 5, 5), (2, 10, 3))   # dx'=-1 col, dx'=+1 col, dx'=0 dy' even
PE_GROUPS3 = ((0, 3, 3), (2, 3, 3))               # dx'=-1 col, dx'=+1 col


def _taps_of(groups, kk):
    out = []
    for start, step, count in groups:
        for a in range(count):
            t = start + a * step
            out.append((t // kk - kk // 2, t % kk - kk // 2))
    return tuple(out)


PE_TAPS5 = _taps_of(PE_GROUPS5, 5)
PE_TAPS3 = _taps_of(PE_GROUPS3, 3)
CNT5, CNT3, CNT1 = (Hy // 5) ** 2, (Hy // 3) ** 2, Hy * Hy  # 144, 400, 3600
YCH = (Hy * Hy + P - 1) // P  # 29 zero-padded pixel chunks of transposed y

# column layout of the fp32 per-partition bias tile [128, 28]
TRB5, TRB3, GKB1, GKB3, GKB5, BFO, OB = 0, 4, 8, 12, 16, 20, 24

_CACHED = {}


def _build_program():
    import concourse.bacc as bacc
    import concourse.mybir as mybir
    import concourse.tile as tile

    fp16 = mybir.dt.float16
    fp32 = mybir.dt.float32
    Alu = mybir.AluOpType
    AF = mybir.ActivationFunctionType
    AX = mybir.AxisListType

    nc = bacc.Bacc("TRN2", debug=False)

    # ---- DRAM I/O (per-core layouts, partition-major) ----
    din = {}
    def ext_in(name, shape, dt):
        din[name] = nc.dram_tensor(name, shape, dt, kind="ExternalInput").ap()
        return din[name]

    x16 = ext_in("x16", [P, KC, HW], fp16)
    y16 = ext_in("yT", [P, YCH, inC], fp16)
    pmd = ext_in("pm", [P, YCH, 35], fp16)
    trT5 = ext_in("trT5", [P, KC, midC], fp16)
    trT3 = ext_in("trT3", [P, KC, midC], fp16)
    GT5 = ext_in("GT5", [P, KC, outC], fp16)
    GT3 = ext_in("GT3", [P, KC, outC], fp16)
    gkT1 = ext_in("gkT1", [P, KC, midC], fp16)
    gkT3 = ext_in("gkT3", [P, KC, midC], fp16)
    gkT5 = ext_in("gkT5", [P, KC, midC], fp16)
    tr1 = ext_in("tr1", [P, KC, inC], fp16)
    G1T = ext_in("G1T", [P, KC, outC], fp16)
    foxT = ext_in("foxT", [P, KC, outC], fp16)
    b1d = ext_in("b1", [P, KC, 1], fp16)
    biasd = ext_in("biases", [P, 28], fp32)
    eyed = ext_in("eye", [P, P], fp16)

    outd = nc.dram_tensor("out", [P, KC, HW], fp32, kind="ExternalOutput").ap()

    with tile.TileContext(nc) as tc:
        _emit(nc, tc, mybir, din, outd, fp16, fp32, Alu, AF, AX)
    nc.compile()
    return nc


def _emit(nc, tc, mybir, din, outd, fp16, fp32, Alu, AF, AX):
    from contextlib import ExitStack

    ctx = ExitStack()
    with ctx:
        wmain = ctx.enter_context(tc.tile_pool(name="wmain", bufs=1))
        accp = ctx.enter_context(tc.tile_pool(name="accp", bufs=1))
        psum = ctx.enter_context(tc.tile_pool(name="psum", bufs=2, space="PSUM"))

        # ---------- persistent tiles ----------
        xs = wmain.tile([P, KC, HW], fp16, tag="xs")
        kern = wmain.tile([P, KC, 36], fp32, tag="kern")     # [mid-chunk, tap]
        biases = wmain.tile([P, 28], fp32, tag="biases")
        w_trT5 = wmain.tile([P, KC, midC], fp16, tag="trT5")
        w_trT3 = wmain.tile([P, KC, midC], fp16, tag="trT3")
        w_GT5 = wmain.tile([P, KC, outC], fp16, tag="GT5")
        w_GT3 = wmain.tile([P, KC, outC], fp16, tag="GT3")
        w_C1T = wmain.tile([P, KC, outC], fp16, tag="C1T")
        acc5 = accp.tile([P, KC, HW], fp16, tag="acc5")
        acc3 = accp.tile([P, KC, HW], fp16, tag="acc3")

        # ---------- stage 1: pooling + kernel generation + k1 matrix ----------
        with tc.tile_pool(name="wtmp", bufs=1) as wtmp:
            yT = wtmp.tile([P, YCH, inC], fp16, tag="yT")
            pm = wtmp.tile([P, YCH, 35], fp16, tag="pm")
            pooled = wtmp.tile([P, KC, 35], fp16, tag="pooled")
            w_gk = [wtmp.tile([P, KC, midC], fp16, tag=f"gkT{k}", name=f"gkT{k}")
                    for k in (1, 3, 5)]
            w_tr1 = wtmp.tile([P, KC, inC], fp16, tag="tr1")
            w_G1T = wtmp.tile([P, KC, outC], fp16, tag="G1T")
            w_foxT = wtmp.tile([P, KC, outC], fp16, tag="foxT")
            w_Gs1T = wtmp.tile([P, KC, outC], fp16, tag="Gs1T")
            b1 = wtmp.tile([P, KC, 1], fp16, tag="b1")

            # y first: pooling is the head of the critical path. Spread the
            # bulk input DMAs across engine queues so they run in parallel.
            nc.sync.dma_start(pm[:], din["pm"][:])
            nc.sync.dma_start(yT[:], din["yT"][:])
            nc.scalar.dma_start(xs[:], din["x16"][:])
            for t, n in ((w_trT5, "trT5"), (w_trT3, "trT3")):
                nc.scalar.dma_start(t[:], din[n][:])
            for t, n in ((w_GT5, "GT5"), (w_GT3, "GT3"), (biases, "biases")):
                nc.gpsimd.dma_start(t[:], din[n][:])
            for t, n in ((w_gk[0], "gkT1"), (w_gk[1], "gkT3"), (w_gk[2], "gkT5"),
                         (w_tr1, "tr1"), (w_G1T, "G1T"), (w_foxT, "foxT"),
                         (b1, "b1")):
                nc.sync.dma_start(t[:], din[n][:])

            # pooling on TensorE: pooled[c, j] = sum_px yT[px, c] * pm[px, j]
            # (pm carries the 1/count mean normalization and the zero row pad)
            for m in range(KC):
                psp = psum.tile([P, HALF], mybir.dt.float32, tag="ps",
                                name="psp")
                for ch in range(YCH):
                    nc.tensor.matmul(
                        psp[:, :35],
                        yT[:, ch, m * P:(m + 1) * P],
                        pm[:, ch, :],
                        start=(ch == 0), stop=(ch == YCH - 1))
                nc.vector.tensor_copy(pooled[:, m, :], psp[:, :35])

            # kernel-generator matmuls: kern = gk_w @ pooled + gk_b
            for w_g, off, kk2, gb in ((w_gk[2], OFF5, 25, GKB5),
                                      (w_gk[1], OFF3, 9, GKB3),
                                      (w_gk[0], OFF1, 1, GKB1)):
                for m in range(KC):
                    ps = psum.tile([P, HALF], mybir.dt.float32, tag="ps")
                    for kc in range(KC):
                        nc.tensor.matmul(
                            ps[:, :kk2],
                            w_g[:, kc, m * P:(m + 1) * P],
                            pooled[:, kc, off:off + kk2],
                            start=(kc == 0), stop=(kc == KC - 1))
                    nc.scalar.activation(
                        kern[:, m, off:off + kk2], ps[:, :kk2],
                        AF.Identity, bias=biases[:, gb + m:gb + m + 1])

            # k1 branch folded matrix: C1T = foxT + (tr1^T @ (G1T*kern1))
            for kc in range(KC):
                nc.vector.tensor_scalar_mul(
                    w_Gs1T[:, kc, :], w_G1T[:, kc, :], kern[:, kc, OFF1:OFF1 + 1])
            for mi in range(KC):
                ps = psum.tile([P, HALF], mybir.dt.float32, tag="ps")
                for kc in range(KC):
                    nc.tensor.matmul(
                        ps[:, :outC], w_tr1[:, kc, mi * P:(mi + 1) * P],
                        w_Gs1T[:, kc, :],
                        start=(kc == 0), stop=(kc == KC - 1))
                nc.vector.tensor_tensor(
                    w_C1T[:, mi, :], ps[:, :outC], w_foxT[:, mi, :], op=Alu.add)
            # out bias = b' + G1 @ (kern1 * b1)   (v1 via tiny matvec)
            for m in range(KC):
                ps = psum.tile([P, HALF], mybir.dt.float32, tag="ps")
                for kc in range(KC):
                    nc.tensor.matmul(
                        ps[:, :1], w_Gs1T[:, kc, m * P:(m + 1) * P],
                        b1[:, kc, :],
                        start=(kc == 0), stop=(kc == KC - 1))
                nc.vector.tensor_tensor(
                    biases[:, OB + m:OB + m + 1], ps[:, :1],
                    biases[:, BFO + m:BFO + m + 1], op=Alu.add)

        # ---------- stage 2+3: branches (tr + depthwise) and fused fo ----------
        # Depthwise split: PE taps run on TensorE as diag(kern) matmuls
        # accumulating in PSUM (the eviction initializes the accumulator);
        # the rest run on VectorE as tensor_scalar(4x) + tensor_tensor(2x).
        eye = wmain.tile([P, P], fp16, tag="eye")
        nc.sync.dma_start(eye[:], din["eye"][:])
        n_pe_max = max(len(PE_TAPS5), len(PE_TAPS3))

        # C1 @ x early (depends only on pooling chain, not on dw):
        # partial = C1.T@x + out_bias, stored fp16; the final fo eviction
        # adds it on VectorE (idle in the tail).
        fop = ctx.enter_context(tc.tile_pool(name="fop", bufs=1))
        partial = fop.tile([P, KC, HW], fp16, tag="partial")
        for m in range(KC):
            for half in range(2):
                ps = psum.tile([P, HALF], mybir.dt.float32, tag="ps",
                               name="psc1")
                for kc in range(KC):
                    for nb in range(HALF // NBLK):
                        nc.tensor.matmul(
                            ps[:, nb * NBLK:(nb + 1) * NBLK],
                            w_C1T[:, kc, m * P:(m + 1) * P],
                            xs[:, kc, half * HALF + nb * NBLK:
                               half * HALF + (nb + 1) * NBLK],
                            start=(kc == 0), stop=(kc == KC - 1))
                nc.scalar.activation(
                    partial[:, m, half * HALF:(half + 1) * HALF], ps[:],
                    AF.Identity, bias=biases[:, OB + m:OB + m + 1])

        with tc.tile_pool(name="pads", bufs=4) as pads, \
             tc.tile_pool(name="dtmp", bufs=1) as dtmp, \
             tc.tile_pool(name="diagp", bufs=2) as diagp, \
             tc.tile_pool(name="outp", bufs=2) as outp:
            br5 = (w_trT5, acc5, OFF5, 5, TRB5, PE_TAPS5, PE_GROUPS5)
            br3 = (w_trT3, acc3, OFF3, 3, TRB3, PE_TAPS3, PE_GROUPS3)
            units = [(br5, m) for m in range(KC)]
            units += [("fo5", None)]
            units += [(br3, m) for m in range(KC)]
            for br, m in units:
                if br == "fo5":
                    # acc5 complete: fold G5 @ acc5 into the fp16 partial now
                    # (fills the PE while the k3 units' DVE taps run)
                    for mo in range(KC):
                        for half in range(2):
                            ps = psum.tile([P, HALF], mybir.dt.float32,
                                           tag="ps", name="ps5")
                            for kc in range(KC):
                                for nb in range(HALF // NBLK):
                                    nc.tensor.matmul(
                                        ps[:, nb * NBLK:(nb + 1) * NBLK],
                                        w_GT5[:, kc, mo * P:(mo + 1) * P],
                                        acc5[:, kc, half * HALF + nb * NBLK:
                                             half * HALF + (nb + 1) * NBLK],
                                        start=(kc == 0), stop=(kc == KC - 1))
                            pslice = partial[:, mo, half * HALF:(half + 1) * HALF]
                            nc.vector.tensor_tensor(
                                pslice, ps[:], pslice, op=Alu.add)
                    continue
                w_tr, acc, koff, kk, trb, pe_taps, pe_groups = br
                p = kk // 2
                dve_taps = [(dy, dx) for dy in range(-p, p + 1)
                            for dx in range(-p, p + 1)
                            if (dy, dx) not in pe_taps]
                # build this unit's diag(kern) matrices (one op per group)
                diags = diagp.tile([P, n_pe_max, P], fp16, tag="diags")
                di = 0
                diag_idx = {}
                for start, step, count in pe_groups:
                    kv = kern[:, m, koff + start: koff + start + step * count]
                    kv = kv.rearrange("p (a b) -> p a b", b=step)[:, :, 0:1]
                    nc.vector.tensor_tensor(
                        diags[:, di:di + count, :],
                        eye[:].rearrange("p (o a) -> p o a", o=1).to_broadcast(
                            [P, count, P]),
                        kv.to_broadcast([P, count, P]),
                        op=Alu.mult)
                    for a in range(count):
                        t = start + a * step
                        diag_idx[(koff, m, t // kk - p, t % kk - p)] = di + a
                    di += count
                xpad = pads.tile([P, PR, PCE], fp16, tag="xpad")
                # zero the halo borders (interior is fully overwritten)
                nc.gpsimd.memset(xpad[:, 0:PAD, :], 0.0)
                nc.gpsimd.memset(xpad[:, PAD + H:PR, :], 0.0)
                nc.gpsimd.memset(xpad[:, PAD:PAD + H, 0:PAD], 0.0)
                nc.gpsimd.memset(xpad[:, PAD:PAD + H, PCE - PAD:PCE], 0.0)
                av = acc[:, m].rearrange("p (h w) -> p h w", w=W)
                for half in range(2):
                    ps = psum.tile([P, HALF], mybir.dt.float32, tag="ps")
                    for kc in range(KC):
                        for nb in range(HALF // NBLK):
                            nc.tensor.matmul(
                                ps[:, nb * NBLK:(nb + 1) * NBLK],
                                w_tr[:, kc, m * P:(m + 1) * P],
                                xs[:, kc, half * HALF + nb * NBLK:
                                   half * HALF + (nb + 1) * NBLK],
                                start=(kc == 0), stop=(kc == KC - 1))
                    psv = ps[:].rearrange("p (r c) -> p r c", c=W)
                    r0 = PAD + half * (H // 2)
                    nc.scalar.activation(
                        xpad[:, r0:r0 + H // 2, PAD:PAD + W], psv,
                        AF.Identity, bias=biases[:, trb + m:trb + m + 1])
                # PE taps: diag matmuls accumulate in PSUM, evict = init
                for half in range(2):
                    psd = psum.tile([P, HALF], mybir.dt.float32, tag="ps",
                                    name="psd")
                    for ti, (dy, dx) in enumerate(pe_taps):
                        dg = diags[:, diag_idx[(koff, m, dy, dx)], :]
                        for nb in range(4):
                            r0 = half * 32 + nb * 8
                            nc.tensor.matmul(
                                psd[:, nb * NBLK:(nb + 1) * NBLK],
                                dg,
                                xpad[:, PAD + dy + r0:PAD + dy + r0 + 8,
                                     PAD + dx:PAD + dx + W],
                                start=(ti == 0), stop=(ti == len(pe_taps) - 1))
                    nc.scalar.copy(
                        av[:, half * 32:half * 32 + 32, :],
                        psd[:].rearrange("p (r c) -> p r c", c=W))
                # DVE taps: mul at 4x into tmp, add at 2x into acc
                for (dy, dx) in dve_taps:
                    tap = koff + (dy + p) * kk + (dx + p)
                    tmp = dtmp.tile([P, HW], fp16, tag="dvetmp")
                    tv = tmp[:].rearrange("p (h w) -> p h w", w=W)
                    nc.vector.tensor_scalar_mul(
                        tv, xpad[:, PAD + dy:PAD + dy + H,
                                 PAD + dx:PAD + dx + W],
                        kern[:, m, tap:tap + 1])
                    nc.vector.tensor_tensor(av, tv, av, op=Alu.add)

            # ---------- tail: fo over the k3 dw outputs + partial add ----------
            mats_fo = ((w_GT3, acc3),)
            for m in range(KC):
                for half in range(2):
                    ps = psum.tile([P, HALF], mybir.dt.float32, tag="ps",
                                   name="psfo")
                    for mi, (wm, rhs) in enumerate(mats_fo):
                        for kc in range(KC):
                            for nb in range(HALF // NBLK):
                                nc.tensor.matmul(
                                    ps[:, nb * NBLK:(nb + 1) * NBLK],
                                    wm[:, kc, m * P:(m + 1) * P],
                                    rhs[:, kc, half * HALF + nb * NBLK:
                                        half * HALF + (nb + 1) * NBLK],
                                    start=(mi == 0 and kc == 0),
                                    stop=(mi == len(mats_fo) - 1
                                          and kc == KC - 1))
                    for q in range(2):
                        o0 = half * HALF + q * (HALF // 2)
                        outs = outp.tile([P, HALF // 2], fp32, tag="outs")
                        nc.vector.tensor_tensor(
                            outs[:], ps[:, q * (HALF // 2):(q + 1) * (HALF // 2)],
                            partial[:, m, o0:o0 + HALF // 2],
                            op=Alu.add)
                        nc.sync.dma_start(
                            outd[:, m, o0:o0 + HALF // 2], outs[:])


def _chunk_pm(a):
    """[512, F...] -> [128, 4, F...] partition-major chunks."""
    return np.ascontiguousarray(
        a.reshape(KC, P, *a.shape[1:]).transpose(1, 0, *range(2, a.ndim + 1)))


def _prep_host(inputs):
    """Host-side weight folding + layout. Returns (shared weight arrays,
    per-core input arrays)."""
    f32 = np.float32
    x = np.asarray(inputs["x"], f32)
    y = np.asarray(inputs["y"], f32)
    gk_w = np.asarray(inputs["gk_w"], f32)
    gk_b = np.asarray(inputs["gk_b"], f32)
    tr_w = np.asarray(inputs["tr_w"], f32)
    tr_b = np.asarray(inputs["tr_b"], f32)
    fi_w = np.asarray(inputs["fi_w"], f32)
    fi_b = np.asarray(inputs["fi_b"], f32)
    fo_w = np.asarray(inputs["fo_w"], f32)
    fo_b = np.asarray(inputs["fo_b"], f32)

    fo_x = fo_w[:, :inC]
    fo_blk = [fo_w[:, inC + i * midC: inC + (i + 1) * midC] for i in range(3)]
    G = [fo_blk[i] @ fi_w[i] for i in range(3)]            # [outC, midC]
    bfo = fo_b + sum(fo_blk[i] @ fi_b[i] for i in range(3))

    f16 = np.float16
    shared = {
        "trT5": _chunk_pm(np.ascontiguousarray(tr_w[2].T).astype(f16)),
        "trT3": _chunk_pm(np.ascontiguousarray(tr_w[1].T).astype(f16)),
        "GT5": _chunk_pm(np.ascontiguousarray(G[2].T).astype(f16)),
        "GT3": _chunk_pm(np.ascontiguousarray(G[1].T).astype(f16)),
        "gkT1": _chunk_pm(np.ascontiguousarray(gk_w[0].T).astype(f16)),
        "gkT3": _chunk_pm(np.ascontiguousarray(gk_w[1].T).astype(f16)),
        "gkT5": _chunk_pm(np.ascontiguousarray(gk_w[2].T).astype(f16)),
        "tr1": _chunk_pm(tr_w[0].astype(f16)),
        "G1T": _chunk_pm(np.ascontiguousarray(G[0].T).astype(f16)),
        "foxT": _chunk_pm(np.ascontiguousarray(fo_x.T).astype(f16)),
        "b1": _chunk_pm(tr_b[0].astype(f16)[:, None]),
        "eye": np.eye(P, dtype=f16),
    }
    # pooling matrix [3600-padded, 35]: col j sums pixels of block j, scaled
    # by 1/count so the matmul produces the block means directly
    pmat = np.zeros((YCH * P, 35), np.float32)
    hw_idx = np.arange(Hy * Hy)
    hh, ww = hw_idx // Hy, hw_idx % Hy
    for j in range(25):
        jh, jw = j // 5, j % 5
        pmat[:Hy * Hy, OFF5 + j] = ((hh // (Hy // 5) == jh) &
                                    (ww // (Hy // 5) == jw)) / CNT5
    for j in range(9):
        jh, jw = j // 3, j % 3
        pmat[:Hy * Hy, OFF3 + j] = ((hh // (Hy // 3) == jh) &
                                    (ww // (Hy // 3) == jw)) / CNT3
    pmat[:Hy * Hy, OFF1] = 1.0 / CNT1
    shared["pm"] = np.ascontiguousarray(
        pmat.reshape(YCH, P, 35).transpose(1, 0, 2)).astype(f16)
    biases = np.zeros((P, 28), f32)
    for col, vec in ((TRB5, tr_b[2]), (TRB3, tr_b[1]), (GKB1, gk_b[0]),
                     (GKB3, gk_b[1]), (GKB5, gk_b[2]), (BFO, bfo)):
        biases[:, col:col + KC] = vec.reshape(KC, P).T
    shared["biases"] = biases

    per_core = []
    for b in range(B):
        yt = np.zeros((YCH * P, inC), f16)
        yt[:Hy * Hy] = y[b].reshape(inC, Hy * Hy).T.astype(f16)
        per_core.append({
            "x16": _chunk_pm(x[b].reshape(inC, HW).astype(f16)),
            "yT": np.ascontiguousarray(
                yt.reshape(YCH, P, inC).transpose(1, 0, 2)),
        })
    return shared, per_core


LAST_RESULTS = None


def _ensure_ntff_hook():
    """Best-effort: recreate the missing antenv.axon_hooks module so
    run_bass_kernel_spmd(trace=True) can capture NTFF profiles under axon."""
    import sys
    import types
    try:
        from antenv.axon_hooks import get_axon_ntff_profile_hook  # noqa: F401
        return
    except ImportError:
        pass
    try:
        import antenv
        from trn_agent_boot.trn_boot import _ntff_profile_via_ctypes
        mod = types.ModuleType("antenv.axon_hooks")
        mod._hook = None

        def set_axon_ntff_profile_hook(h):
            mod._hook = h

        def get_axon_ntff_profile_hook():
            return mod._hook

        mod.set_axon_ntff_profile_hook = set_axon_ntff_profile_hook
        mod.get_axon_ntff_profile_hook = get_axon_ntff_profile_hook
        sys.modules["antenv.axon_hooks"] = mod
        antenv.axon_hooks = mod
        mod.set_axon_ntff_profile_hook(
            _ntff_profile_via_ctypes("/opt/axon/libaxon_pjrt.so"))
    except Exception as e:  # profiling is optional — never break the run
        print(f"ntff hook unavailable: {e}")


def kernel(**inputs) -> np.ndarray:
    global LAST_RESULTS
    if "nc" not in _CACHED:
        _CACHED["nc"] = _build_program()
    nc = _CACHED["nc"]

    shared, per_core = _prep_host(inputs)
    in_maps = [{**shared, **pc} for pc in per_core]

    from concourse import bass_utils
    trace = bool(os.environ.get("DCM_TRACE"))
    if trace:
        _ensure_ntff_hook()
    res = bass_utils.run_bass_kernel_spmd(
        nc, in_maps, core_ids=list(range(N_CORES)), trace=trace)
    LAST_RESULTS = res

    out = np.empty((B, outC, H, W), np.float32)
    for b in range(B):
        o = res.results[b]["out"]                      # [128, KC, HW]
        out[b] = o.transpose(1, 0, 2).reshape(outC, H, W)
    return out


# revision 36
# speedup vs baseline: 1.0551x; 1.0551x over previous
"""Trainium2 Bass kernel for nn_DCM_22058952032783 (dynamic-conv CNN).

Strategy: pure data-parallel over batch (B=8 -> 8 NeuronCores, one sample
per core, no collectives).

Per-sample math (reference):
    feats = [x]
    for k in (1, 3, 5):
        pooled = adaptive_avg_pool(y, k)               # [inC, k, k]
        kern   = gk_w @ pooled_mean + gk_b             # [mid, k*k] per-sample dw kernels
        x_in   = tr_w @ x + tr_b                       # [mid, HW]
        dwout  = depthwise(x_in, kern)                 # [mid, HW]
        feats.append(fi_w @ dwout + fi_b)
    out = fo_w @ concat(feats) + fo_b

Host-side exact algebraic folds (weights-only preprocessing):
  - G_i  = fo_blk_i @ fi_w_i   folds the fi convs into fo:
        out = fo_x@x + sum_i G_i @ dwout_i + b'   with b' = fo_b + sum fo_blk_i@fi_b_i
  - The k=1 branch is a per-channel scale:  G1 @ (kern1 * (tr1@x + b1)) =
        [G1 diag(kern1) tr1] @ x + G1@(kern1*b1).  The bracket is built on
    device with one small matmul chain and merged with fo_x -> C1.
Device work per core: 3 remaining dense 1x1 conv stacks on TensorE (tr_k3,
tr_k5, and the fused fo pass), depthwise taps as per-partition-scalar FMAs
on VectorE, PSUM evictions on ScalarE, pooling reductions on VectorE.
"""

import os

import numpy as np

# ---- hardcoded problem shapes (nn_DCM_22058952032783) ----
B, inC, midC, outC, H, W, Hy = 8, 512, 512, 512, 64, 64, 60
HW = H * W            # 4096
P = 128
KC = inC // P         # 4 partition chunks of the channel dims
NBLK = 512            # matmul moving free dim
HALF = 2048           # psum half (4 banks)
PAD = 2
PR = H + 2 * PAD      # 68 padded rows
PCE = W + 2 * PAD     # 68 cols, even-aligned pad buffer
PCO = W + 2           # 66 cols, odd-shifted pad buffer
N_CORES = 8

# column layout of the fused pooled/kern tiles: [k5 (25) | k3 (9) | k1 (1)]
OFF5, OFF3, OFF1 = 0, 25, 34

# depthwise taps routed to TensorE (diag matmuls); the rest go to VectorE.
# Groups are (start, step, count) strided runs of within-branch tap indices so
# each group's diag matrices build with ONE broadcast tensor_tensor op.
PE_GROUPS5 = ((1, 5, 5), (3, 5, 5), (2, 10, 3))   # dx'=-1 col, dx'=+1 col, dx'=0 dy' even
PE_GROUPS3 = ((0, 3, 3), (2, 3, 3))               # dx'=-1 col, dx'=+1 col


def _taps_of(groups, kk):
    out = []
    for start, step, count in groups:
        for a in range(count):
            t = start + a * step
            out.append((t // kk - kk // 2, t % kk - kk // 2))
    return tuple(out)


PE_TAPS5 = _taps_of(PE_GROUPS5, 5)
PE_TAPS3 = _taps_of(PE_GROUPS3, 3)
CNT5, CNT3, CNT1 = (Hy // 5) ** 2, (Hy // 3) ** 2, Hy * Hy  # 144, 400, 3600
YCH = (Hy * Hy + P - 1) // P  # 29 zero-padded pixel chunks of transposed y

# column layout of the fp32 per-partition bias tile [128, 28]
TRB5, TRB3, GKB1, GKB3, GKB5, BFO, OB = 0, 4, 8, 12, 16, 20, 24

_CACHED = {}


def _build_program():
    import concourse.bacc as bacc
    import concourse.mybir as mybir
    import concourse.tile as tile

    fp16 = mybir.dt.float16
    fp32 = mybir.dt.float32
    Alu = mybir.AluOpType
    AF = mybir.ActivationFunctionType
    AX = mybir.AxisListType

    nc = bacc.Bacc("TRN2", debug=False)

    # ---- DRAM I/O (per-core layouts, partition-major) ----
    din = {}
    def ext_in(name, shape, dt):
        din[name] = nc.dram_tensor(name, shape, dt, kind="ExternalInput").ap()
        return din[name]

    x16 = ext_in("x16", [P, KC, HW], fp16)
    y16 = ext_in("yT", [P, YCH, inC], fp16)
    pmd = ext_in("pm", [P, YCH, 35], fp16)
    trT5 = ext_in("trT5", [P, KC, midC], fp16)
    trT3 = ext_in("trT3", [P, KC, midC], fp16)
    GT5 = ext_in("GT5", [P, KC, outC], fp16)
    GT3 = ext_in("GT3", [P, KC, outC], fp16)
    gkT1 = ext_in("gkT1", [P, KC, midC], fp16)
    gkT3 = ext_in("gkT3", [P, KC, midC], fp16)
    gkT5 = ext_in("gkT5", [P, KC, midC], fp16)
    tr1 = ext_in("tr1", [P, KC, inC], fp16)
    G1T = ext_in("G1T", [P, KC, outC], fp16)
    foxT = ext_in("foxT", [P, KC, outC], fp16)
    b1d = ext_in("b1", [P, KC, 1], fp16)
    biasd = ext_in("biases", [P, 28], fp32)
    eyed = ext_in("eye", [P, P], fp16)

    outd = nc.dram_tensor("out", [P, KC, HW], fp32, kind="ExternalOutput").ap()

    with tile.TileContext(nc) as tc:
        _emit(nc, tc, mybir, din, outd, fp16, fp32, Alu, AF, AX)
    nc.compile()
    return nc


def _emit(nc, tc, mybir, din, outd, fp16, fp32, Alu, AF, AX):
    from contextlib import ExitStack

    ctx = ExitStack()
    with ctx:
        wmain = ctx.enter_context(tc.tile_pool(name="wmain", bufs=1))
        accp = ctx.enter_context(tc.tile_pool(name="accp", bufs=1))
        psum = ctx.enter_context(tc.tile_pool(name="psum", bufs=2, space="PSUM"))

        # ---------- persistent tiles ----------
        xs = wmain.tile([P, KC, HW], fp16, tag="xs")
        kern = wmain.tile([P, KC, 36], fp32, tag="kern")     # [mid-chunk, tap]
        biases = wmain.tile([P, 28], fp32, tag="biases")
        w_trT5 = wmain.tile([P, KC, midC], fp16, tag="trT5")
        w_trT3 = wmain.tile([P, KC, midC], fp16, tag="trT3")
        w_GT5 = wmain.tile([P, KC, outC], fp16, tag="GT5")
        w_GT3 = wmain.tile([P, KC, outC], fp16, tag="GT3")
        w_C1T = wmain.tile([P, KC, outC], fp16, tag="C1T")
        acc5 = accp.tile([P, KC, HW], fp16, tag="acc5")
        acc3 = accp.tile([P, KC, HW], fp16, tag="acc3")

        # ---------- stage 1: pooling + kernel generation + k1 matrix ----------
        with tc.tile_pool(name="wtmp", bufs=1) as wtmp:
            yT = wtmp.tile([P, YCH, inC], fp16, tag="yT")
            pm = wtmp.tile([P, YCH, 35], fp16, tag="pm")
            pooled = wtmp.tile([P, KC, 35], fp16, tag="pooled")
            w_gk = [wtmp.tile([P, KC, midC], fp16, tag=f"gkT{k}", name=f"gkT{k}")
                    for k in (1, 3, 5)]
            w_tr1 = wtmp.tile([P, KC, inC], fp16, tag="tr1")
            w_G1T = wtmp.tile([P, KC, outC], fp16, tag="G1T")
            w_foxT = wtmp.tile([P, KC, outC], fp16, tag="foxT")
            w_Gs1T = wtmp.tile([P, KC, outC], fp16, tag="Gs1T")
            b1 = wtmp.tile([P, KC, 1], fp16, tag="b1")

            # y first: pooling is the head of the critical path. Spread the
            # bulk input DMAs across engine queues so they run in parallel.
            nc.sync.dma_start(pm[:], din["pm"][:])
            nc.sync.dma_start(yT[:], din["yT"][:])
            nc.scalar.dma_start(xs[:], din["x16"][:])
            for t, n in ((w_trT5, "trT5"), (w_trT3, "trT3")):
                nc.scalar.dma_start(t[:], din[n][:])
            for t, n in ((w_GT5, "GT5"), (w_GT3, "GT3"), (biases, "biases")):
                nc.gpsimd.dma_start(t[:], din[n][:])
            for t, n in ((w_gk[0], "gkT1"), (w_gk[1], "gkT3"), (w_gk[2], "gkT5"),
                         (w_tr1, "tr1"), (w_G1T, "G1T"), (w_foxT, "foxT"),
                         (b1, "b1")):
                nc.sync.dma_start(t[:], din[n][:])

            # pooling on TensorE: pooled[c, j] = sum_px yT[px, c] * pm[px, j]
            # (pm carries the 1/count mean normalization and the zero row pad)
            for m in range(KC):
                psp = psum.tile([P, HALF], mybir.dt.float32, tag="ps",
                                name="psp")
                for ch in range(YCH):
                    nc.tensor.matmul(
                        psp[:, :35],
                        yT[:, ch, m * P:(m + 1) * P],
                        pm[:, ch, :],
                        start=(ch == 0), stop=(ch == YCH - 1))
                nc.vector.tensor_copy(pooled[:, m, :], psp[:, :35])

            # kernel-generator matmuls: kern = gk_w @ pooled + gk_b
            for w_g, off, kk2, gb in ((w_gk[2], OFF5, 25, GKB5),
                                      (w_gk[1], OFF3, 9, GKB3),
                                      (w_gk[0], OFF1, 1, GKB1)):
                for m in range(KC):
                    ps = psum.tile([P, HALF], mybir.dt.float32, tag="ps")
                    for kc in range(KC):
                        nc.tensor.matmul(
                            ps[:, :kk2],
                            w_g[:, kc, m * P:(m + 1) * P],
                            pooled[:, kc, off:off + kk2],
                            start=(kc == 0), stop=(kc == KC - 1))
                    nc.scalar.activation(
                        kern[:, m, off:off + kk2], ps[:, :kk2],
                        AF.Identity, bias=biases[:, gb + m:gb + m + 1])

            # k1 branch folded matrix: C1T = foxT + (tr1^T @ (G1T*kern1))
            for kc in range(KC):
                nc.vector.tensor_scalar_mul(
                    w_Gs1T[:, kc, :], w_G1T[:, kc, :], kern[:, kc, OFF1:OFF1 + 1])
            for mi in range(KC):
                ps = psum.tile([P, HALF], mybir.dt.float32, tag="ps")
                for kc in range(KC):
                    nc.tensor.matmul(
                        ps[:, :outC], w_tr1[:, kc, mi * P:(mi + 1) * P],
                        w_Gs1T[:, kc, :],
                        start=(kc == 0), stop=(kc == KC - 1))
                nc.vector.tensor_tensor(
                    w_C1T[:, mi, :], ps[:, :outC], w_foxT[:, mi, :], op=Alu.add)
            # out bias = b' + G1 @ (kern1 * b1)   (v1 via tiny matvec)
            for m in range(KC):
                ps = psum.tile([P, HALF], mybir.dt.float32, tag="ps")
                for kc in range(KC):
                    nc.tensor.matmul(
                        ps[:, :1], w_Gs1T[:, kc, m * P:(m + 1) * P],
                        b1[:, kc, :],
                        start=(kc == 0), stop=(kc == KC - 1))
                nc.vector.tensor_tensor(
                    biases[:, OB + m:OB + m + 1], ps[:, :1],
                    biases[:, BFO + m:BFO + m + 1], op=Alu.add)

        # ---------- stage 2+3: branches (tr + depthwise) and fused fo ----------
        # Depthwise split: PE taps run on TensorE as diag(kern) matmuls
        # accumulating in PSUM (the eviction initializes the accumulator);
        # the rest run on VectorE as tensor_scalar(4x) + tensor_tensor(2x).
        eye = wmain.tile([P, P], fp16, tag="eye")
        nc.sync.dma_start(eye[:], din["eye"][:])
        n_pe_max = max(len(PE_TAPS5), len(PE_TAPS3))

        # C1 @ x early (depends only on pooling chain, not on dw):
        # partial = C1.T@x + out_bias, stored fp16; the final fo eviction
        # adds it on VectorE (idle in the tail).
        fop = ctx.enter_context(tc.tile_pool(name="fop", bufs=1))
        partial = fop.tile([P, KC, HW], fp16, tag="partial")
        for m in range(KC):
            for half in range(2):
                ps = psum.tile([P, HALF], mybir.dt.float32, tag="ps",
                               name="psc1")
                for kc in range(KC):
                    for nb in range(HALF // NBLK):
                        nc.tensor.matmul(
                            ps[:, nb * NBLK:(nb + 1) * NBLK],
                            w_C1T[:, kc, m * P:(m + 1) * P],
                            xs[:, kc, half * HALF + nb * NBLK:
                               half * HALF + (nb + 1) * NBLK],
                            start=(kc == 0), stop=(kc == KC - 1))
                nc.scalar.activation(
                    partial[:, m, half * HALF:(half + 1) * HALF], ps[:],
                    AF.Identity, bias=biases[:, OB + m:OB + m + 1])

        with tc.tile_pool(name="pads", bufs=4) as pads, \
             tc.tile_pool(name="dtmp", bufs=1) as dtmp, \
             tc.tile_pool(name="diagp", bufs=2) as diagp, \
             tc.tile_pool(name="outp", bufs=2) as outp:
            br5 = (w_trT5, acc5, OFF5, 5, TRB5, PE_TAPS5, PE_GROUPS5)
            br3 = (w_trT3, acc3, OFF3, 3, TRB3, PE_TAPS3, PE_GROUPS3)
            units = []
            for m in range(KC - 1):
                units += [(br5, m), (br3, m)]
            units += [(br5, KC - 1), ("fo5", None), (br3, KC - 1)]
            for br, m in units:
                if br == "fo5":
                    # acc5 complete: fold G5 @ acc5 into the fp16 partial now
                    # (fills the PE while the k3 units' DVE taps run)
                    for mo in range(KC):
                        for half in range(2):
                            ps = psum.tile([P, HALF], mybir.dt.float32,
                                           tag="ps", name="ps5")
                            for kc in range(KC):
                                for nb in range(HALF // NBLK):
                                    nc.tensor.matmul(
                                        ps[:, nb * NBLK:(nb + 1) * NBLK],
                                        w_GT5[:, kc, mo * P:(mo + 1) * P],
                                        acc5[:, kc, half * HALF + nb * NBLK:
                                             half * HALF + (nb + 1) * NBLK],
                                        start=(kc == 0), stop=(kc == KC - 1))
                            pslice = partial[:, mo, half * HALF:(half + 1) * HALF]
                            nc.vector.tensor_tensor(
                                pslice, ps[:], pslice, op=Alu.add)
                    continue
                w_tr, acc, koff, kk, trb, pe_taps, pe_groups = br
                p = kk // 2
                dve_taps = [(dy, dx) for dy in range(-p, p + 1)
                            for dx in range(-p, p + 1)
                            if (dy, dx) not in pe_taps]
                # build this unit's diag(kern) matrices (one op per group)
                diags = diagp.tile([P, n_pe_max, P], fp16, tag="diags")
                di = 0
                diag_idx = {}
                for start, step, count in pe_groups:
                    kv = kern[:, m, koff + start: koff + start + step * count]
                    kv = kv.rearrange("p (a b) -> p a b", b=step)[:, :, 0:1]
                    nc.vector.tensor_tensor(
                        diags[:, di:di + count, :],
                        eye[:].rearrange("p (o a) -> p o a", o=1).to_broadcast(
                            [P, count, P]),
                        kv.to_broadcast([P, count, P]),
                        op=Alu.mult)
                    for a in range(count):
                        t = start + a * step
                        diag_idx[(koff, m, t // kk - p, t % kk - p)] = di + a
                    di += count
                xpad = pads.tile([P, PR, PCE], fp16, tag="xpad")
                # zero the halo borders (interior is fully overwritten)
                nc.gpsimd.memset(xpad[:, 0:PAD, :], 0.0)
                nc.gpsimd.memset(xpad[:, PAD + H:PR, :], 0.0)
                nc.gpsimd.memset(xpad[:, PAD:PAD + H, 0:PAD], 0.0)
                nc.gpsimd.memset(xpad[:, PAD:PAD + H, PCE - PAD:PCE], 0.0)
                av = acc[:, m].rearrange("p (h w) -> p h w", w=W)
                for half in range(2):
                    ps = psum.tile([P, HALF], mybir.dt.float32, tag="ps")
                    for kc in range(KC):
                        for nb in range(HALF // NBLK):
                            nc.tensor.matmul(
                                ps[:, nb * NBLK:(nb + 1) * NBLK],
                                w_tr[:, kc, m * P:(m + 1) * P],
                                xs[:, kc, half * HALF + nb * NBLK:
                                   half * HALF + (nb + 1) * NBLK],
                                start=(kc == 0), stop=(kc == KC - 1))
                    psv = ps[:].rearrange("p (r c) -> p r c", c=W)
                    r0 = PAD + half * (H // 2)
                    nc.scalar.activation(
                        xpad[:, r0:r0 + H // 2, PAD:PAD + W], psv,
                        AF.Identity, bias=biases[:, trb + m:trb + m + 1])
                # PE taps: diag matmuls accumulate in PSUM, evict = init
                for half in range(2):
                    psd = psum.tile([P, HALF], mybir.dt.float32, tag="ps",
                                    name="psd")
                    for ti, (dy, dx) in enumerate(pe_taps):
                        dg = diags[:, diag_idx[(koff, m, dy, dx)], :]
                        for nb in range(4):
                            r0 = half * 32 + nb * 8
                            nc.tensor.matmul(
                                psd[:, nb * NBLK:(nb + 1) * NBLK],
                                dg,
                                xpad[:, PAD + dy + r0:PAD + dy + r0 + 8,
                                     PAD + dx:PAD + dx + W],
                                start=(ti == 0), stop=(ti == len(pe_taps) - 1))
                    nc.scalar.copy(
                        av[:, half * 32:half * 32 + 32, :],
                        psd[:].rearrange("p (r c) -> p r c", c=W))
                # DVE taps: mul at 4x into tmp, add at 2x into acc
                for (dy, dx) in dve_taps:
                    tap = koff + (dy + p) * kk + (dx + p)
                    tmp = dtmp.tile([P, HW], fp16, tag="dvetmp")
                    tv = tmp[:].rearrange("p (h w) -> p h w", w=W)
                    nc.vector.tensor_scalar_mul(
                        tv, xpad[:, PAD + dy:PAD + dy + H,
                                 PAD + dx:PAD + dx + W],
                        kern[:, m, tap:tap + 1])
                    nc.vector.tensor_tensor(av, tv, av, op=Alu.add)

            # ---------- tail: fo over the k3 dw outputs + partial add ----------
            mats_fo = ((w_GT3, acc3),)
            for m in range(KC):
                for half in range(2):
                    ps = psum.tile([P, HALF], mybir.dt.float32, tag="ps",
                                   name="psfo")
                    for mi, (wm, rhs) in enumerate(mats_fo):
                        for kc in range(KC):
                            for nb in range(HALF // NBLK):
                                nc.tensor.matmul(
                                    ps[:, nb * NBLK:(nb + 1) * NBLK],
                                    wm[:, kc, m * P:(m + 1) * P],
                                    rhs[:, kc, half * HALF + nb * NBLK:
                                        half * HALF + (nb + 1) * NBLK],
                                    start=(mi == 0 and kc == 0),
                                    stop=(mi == len(mats_fo) - 1
                                          and kc == KC - 1))
                    for q in range(2):
                        o0 = half * HALF + q * (HALF // 2)
                        outs = outp.tile([P, HALF // 2], fp32, tag="outs")
                        nc.vector.tensor_tensor(
                            outs[:], ps[:, q * (HALF // 2):(q + 1) * (HALF // 2)],
                            partial[:, m, o0:o0 + HALF // 2],
                            op=Alu.add)
                        nc.sync.dma_start(
                            outd[:, m, o0:o0 + HALF // 2], outs[:])


def _chunk_pm(a):
    """[512, F...] -> [128, 4, F...] partition-major chunks."""
    return np.ascontiguousarray(
        a.reshape(KC, P, *a.shape[1:]).transpose(1, 0, *range(2, a.ndim + 1)))


def _prep_host(inputs):
    """Host-side weight folding + layout. Returns (shared weight arrays,
    per-core input arrays)."""
    f32 = np.float32
    x = np.asarray(inputs["x"], f32)
    y = np.asarray(inputs["y"], f32)
    gk_w = np.asarray(inputs["gk_w"], f32)
    gk_b = np.asarray(inputs["gk_b"], f32)
    tr_w = np.asarray(inputs["tr_w"], f32)
    tr_b = np.asarray(inputs["tr_b"], f32)
    fi_w = np.asarray(inputs["fi_w"], f32)
    fi_b = np.asarray(inputs["fi_b"], f32)
    fo_w = np.asarray(inputs["fo_w"], f32)
    fo_b = np.asarray(inputs["fo_b"], f32)

    fo_x = fo_w[:, :inC]
    fo_blk = [fo_w[:, inC + i * midC: inC + (i + 1) * midC] for i in range(3)]
    G = [fo_blk[i] @ fi_w[i] for i in range(3)]            # [outC, midC]
    bfo = fo_b + sum(fo_blk[i] @ fi_b[i] for i in range(3))

    f16 = np.float16
    shared = {
        "trT5": _chunk_pm(np.ascontiguousarray(tr_w[2].T).astype(f16)),
        "trT3": _chunk_pm(np.ascontiguousarray(tr_w[1].T).astype(f16)),
        "GT5": _chunk_pm(np.ascontiguousarray(G[2].T).astype(f16)),
        "GT3": _chunk_pm(np.ascontiguousarray(G[1].T).astype(f16)),
        "gkT1": _chunk_pm(np.ascontiguousarray(gk_w[0].T).astype(f16)),
        "gkT3": _chunk_pm(np.ascontiguousarray(gk_w[1].T).astype(f16)),
        "gkT5": _chunk_pm(np.ascontiguousarray(gk_w[2].T).astype(f16)),
        "tr1": _chunk_pm(tr_w[0].astype(f16)),
        "G1T": _chunk_pm(np.ascontiguousarray(G[0].T).astype(f16)),
        "foxT": _chunk_pm(np.ascontiguousarray(fo_x.T).astype(f16)),
        "b1": _chunk_pm(tr_b[0].astype(f16)[:, None]),
        "eye": np.eye(P, dtype=f16),
    }
    # pooling matrix [3600-padded, 35]: col j sums pixels of block j, scaled
    # by 1/count so the matmul produces the block means directly
    pmat = np.zeros((YCH * P, 35), np.float32)
    hw_idx = np.arange(Hy * Hy)
    hh, ww = hw_idx // Hy, hw_idx % Hy
    for j in range(25):
        jh, jw = j // 5, j % 5
        pmat[:Hy * Hy, OFF5 + j] = ((hh // (Hy // 5) == jh) &
                                    (ww // (Hy // 5) == jw)) / CNT5
    for j in range(9):
        jh, jw = j // 3, j % 3
        pmat[:Hy * Hy, OFF3 + j] = ((hh // (Hy // 3) == jh) &
                                    (ww // (Hy // 3) == jw)) / CNT3
    pmat[:Hy * Hy, OFF1] = 1.0 / CNT1
    shared["pm"] = np.ascontiguousarray(
        pmat.reshape(YCH, P, 35).transpose(1, 0, 2)).astype(f16)
    biases = np.zeros((P, 28), f32)
    for col, vec in ((TRB5, tr_b[2]), (TRB3, tr_b[1]), (GKB1, gk_b[0]),
                     (GKB3, gk_b[1]), (GKB5, gk_b[2]), (BFO, bfo)):
        biases[:, col:col + KC] = vec.reshape(KC, P).T
    shared["biases"] = biases

    per_core = []
    for b in range(B):
        yt = np.zeros((YCH * P, inC), f16)
        yt[:Hy * Hy] = y[b].reshape(inC, Hy * Hy).T.astype(f16)
        per_core.append({
            "x16": _chunk_pm(x[b].reshape(inC, HW).astype(f16)),
            "yT": np.ascontiguousarray(
                yt.reshape(YCH, P, inC).transpose(1, 0, 2)),
        })
    return shared, per_core


LAST_RESULTS = None


def _ensure_ntff_hook():
    """Best-effort: recreate the missing antenv.axon_hooks module so
    run_bass_kernel_spmd(trace=True) can capture NTFF profiles under axon."""
    import sys
    import types
    try:
        from antenv.axon_hooks import get_axon_ntff_profile_hook  # noqa: F401
        return
    except ImportError:
        pass
    try:
        import antenv
        from trn_agent_boot.trn_boot import _ntff_profile_via_ctypes
        mod = types.ModuleType("antenv.axon_hooks")
        mod._hook = None

        def set_axon_ntff_profile_hook(h):
            mod._hook = h

        def get_axon_ntff_profile_hook():
            return mod._hook

        mod.set_axon_ntff_profile_hook = set_axon_ntff_profile_hook
        mod.get_axon_ntff_profile_hook = get_axon_ntff_profile_hook
        sys.modules["antenv.axon_hooks"] = mod
        antenv.axon_hooks = mod
        mod.set_axon_ntff_profile_hook(
            _ntff_profile_via_ctypes("/opt/axon/libaxon_pjrt.so"))
    except Exception as e:  # profiling is optional — never break the run
        print(f"ntff hook unavailable: {e}")


def kernel(**inputs) -> np.ndarray:
    global LAST_RESULTS
    if "nc" not in _CACHED:
        _CACHED["nc"] = _build_program()
    nc = _CACHED["nc"]

    shared, per_core = _prep_host(inputs)
    in_maps = [{**shared, **pc} for pc in per_core]

    from concourse import bass_utils
    trace = bool(os.environ.get("DCM_TRACE"))
    if trace:
        _ensure_ntff_hook()
    res = bass_utils.run_bass_kernel_spmd(
        nc, in_maps, core_ids=list(range(N_CORES)), trace=trace)
    LAST_RESULTS = res

    out = np.empty((B, outC, H, W), np.float32)
    for b in range(B):
        o = res.results[b]["out"]                      # [128, KC, HW]
        out[b] = o.transpose(1, 0, 2).reshape(outC, H, W)
    return out


# revision 38
# speedup vs baseline: 1.1482x; 1.0882x over previous
"""Trainium2 Bass kernel for nn_DCM_22058952032783 (dynamic-conv CNN).

Strategy: pure data-parallel over batch (B=8 -> 8 NeuronCores, one sample
per core, no collectives).

Per-sample math (reference):
    feats = [x]
    for k in (1, 3, 5):
        pooled = adaptive_avg_pool(y, k)               # [inC, k, k]
        kern   = gk_w @ pooled_mean + gk_b             # [mid, k*k] per-sample dw kernels
        x_in   = tr_w @ x + tr_b                       # [mid, HW]
        dwout  = depthwise(x_in, kern)                 # [mid, HW]
        feats.append(fi_w @ dwout + fi_b)
    out = fo_w @ concat(feats) + fo_b

Host-side exact algebraic folds (weights-only preprocessing):
  - G_i  = fo_blk_i @ fi_w_i   folds the fi convs into fo:
        out = fo_x@x + sum_i G_i @ dwout_i + b'   with b' = fo_b + sum fo_blk_i@fi_b_i
  - The k=1 branch is a per-channel scale:  G1 @ (kern1 * (tr1@x + b1)) =
        [G1 diag(kern1) tr1] @ x + G1@(kern1*b1).  The bracket is built on
    device with one small matmul chain and merged with fo_x -> C1.
Device work per core: 3 remaining dense 1x1 conv stacks on TensorE (tr_k3,
tr_k5, and the fused fo pass), depthwise taps as per-partition-scalar FMAs
on VectorE, PSUM evictions on ScalarE, pooling reductions on VectorE.
"""

import os

import numpy as np

# ---- hardcoded problem shapes (nn_DCM_22058952032783) ----
B, inC, midC, outC, H, W, Hy = 8, 512, 512, 512, 64, 64, 60
HW = H * W            # 4096
P = 128
KC = inC // P         # 4 partition chunks of the channel dims
NBLK = 512            # matmul moving free dim
HALF = 2048           # psum half (4 banks)
PAD = 2
PR = H + 2 * PAD      # 68 padded rows
PCE = W + 2 * PAD     # 68 cols, even-aligned pad buffer
PCO = W + 2           # 66 cols, odd-shifted pad buffer
N_CORES = 8

# column layout of the fused pooled/kern tiles: [k5 (25) | k3 (9) | k1 (1)]
OFF5, OFF3, OFF1 = 0, 25, 34

# depthwise taps routed to TensorE (diag matmuls); the rest go to VectorE.
# Groups are (start, step, count) strided runs of within-branch tap indices so
# each group's diag matrices build with ONE broadcast tensor_tensor op.
PE_GROUPS5 = ((1, 5, 5), (3, 5, 5), (2, 10, 3))   # dx'=-1 col, dx'=+1 col, dx'=0 dy' even
PE_GROUPS3 = ((0, 3, 3), (2, 3, 3))               # dx'=-1 col, dx'=+1 col


def _taps_of(groups, kk):
    out = []
    for start, step, count in groups:
        for a in range(count):
            t = start + a * step
            out.append((t // kk - kk // 2, t % kk - kk // 2))
    return tuple(out)


PE_TAPS5 = _taps_of(PE_GROUPS5, 5)
PE_TAPS3 = _taps_of(PE_GROUPS3, 3)
CNT5, CNT3, CNT1 = (Hy // 5) ** 2, (Hy // 3) ** 2, Hy * Hy  # 144, 400, 3600
YCH = (Hy * Hy + P - 1) // P  # 29 zero-padded pixel chunks of transposed y

# column layout of the fp32 per-partition bias tile [128, 28]
TRB5, TRB3, GKB1, GKB3, GKB5, BFO, OB = 0, 4, 8, 12, 16, 20, 24

_CACHED = {}


def _build_program():
    import concourse.bacc as bacc
    import concourse.mybir as mybir
    import concourse.tile as tile

    fp16 = mybir.dt.float16
    fp32 = mybir.dt.float32
    Alu = mybir.AluOpType
    AF = mybir.ActivationFunctionType
    AX = mybir.AxisListType

    nc = bacc.Bacc("TRN2", debug=False)

    # ---- DRAM I/O (per-core layouts, partition-major) ----
    din = {}
    def ext_in(name, shape, dt):
        din[name] = nc.dram_tensor(name, shape, dt, kind="ExternalInput").ap()
        return din[name]

    x16 = ext_in("x16", [P, KC, HW], fp16)
    y16 = ext_in("yT", [P, YCH, inC], fp16)
    pmd = ext_in("pm", [P, YCH, 35], fp16)
    trT5 = ext_in("trT5", [P, KC, midC], fp16)
    trT3 = ext_in("trT3", [P, KC, midC], fp16)
    GT5 = ext_in("GT5", [P, KC, outC], fp16)
    GT3 = ext_in("GT3", [P, KC, outC], fp16)
    gkT1 = ext_in("gkT1", [P, KC, midC], fp16)
    gkT3 = ext_in("gkT3", [P, KC, midC], fp16)
    gkT5 = ext_in("gkT5", [P, KC, midC], fp16)
    tr1 = ext_in("tr1", [P, KC, inC], fp16)
    G1T = ext_in("G1T", [P, KC, outC], fp16)
    foxT = ext_in("foxT", [P, KC, outC], fp16)
    b1d = ext_in("b1", [P, KC, 1], fp16)
    biasd = ext_in("biases", [P, 28], fp32)
    eyed = ext_in("eye", [P, P], fp16)

    outd = nc.dram_tensor("out", [P, KC, HW], fp32, kind="ExternalOutput").ap()

    with tile.TileContext(nc) as tc:
        _emit(nc, tc, mybir, din, outd, fp16, fp32, Alu, AF, AX)
    nc.compile()
    return nc


def _emit(nc, tc, mybir, din, outd, fp16, fp32, Alu, AF, AX):
    from contextlib import ExitStack

    ctx = ExitStack()
    with ctx:
        wmain = ctx.enter_context(tc.tile_pool(name="wmain", bufs=1))
        accp = ctx.enter_context(tc.tile_pool(name="accp", bufs=1))
        psum = ctx.enter_context(tc.tile_pool(name="psum", bufs=2, space="PSUM"))

        # ---------- persistent tiles ----------
        xs = wmain.tile([P, KC, HW], fp16, tag="xs")
        kern = wmain.tile([P, KC, 36], fp32, tag="kern")     # [mid-chunk, tap]
        biases = wmain.tile([P, 28], fp32, tag="biases")
        w_trT5 = wmain.tile([P, KC, midC], fp16, tag="trT5")
        w_trT3 = wmain.tile([P, KC, midC], fp16, tag="trT3")
        w_GT5 = wmain.tile([P, KC, outC], fp16, tag="GT5")
        w_GT3 = wmain.tile([P, KC, outC], fp16, tag="GT3")
        w_C1T = wmain.tile([P, KC, outC], fp16, tag="C1T")
        acc5 = accp.tile([P, KC, HW], fp16, tag="acc5")
        acc3 = accp.tile([P, KC, HW], fp16, tag="acc3")

        # ---------- stage 1: pooling + kernel generation + k1 matrix ----------
        with tc.tile_pool(name="wtmp", bufs=1) as wtmp:
            yT = wtmp.tile([P, YCH, inC], fp16, tag="yT")
            pm = wtmp.tile([P, YCH, 35], fp16, tag="pm")
            pooled = wtmp.tile([P, KC, 35], fp16, tag="pooled")
            w_gk = [wtmp.tile([P, KC, midC], fp16, tag=f"gkT{k}", name=f"gkT{k}")
                    for k in (1, 3, 5)]
            w_tr1 = wtmp.tile([P, KC, inC], fp16, tag="tr1")
            w_G1T = wtmp.tile([P, KC, outC], fp16, tag="G1T")
            w_foxT = wtmp.tile([P, KC, outC], fp16, tag="foxT")
            w_Gs1T = wtmp.tile([P, KC, outC], fp16, tag="Gs1T")
            b1 = wtmp.tile([P, KC, 1], fp16, tag="b1")

            # y first: pooling is the head of the critical path. Spread the
            # bulk input DMAs across engine queues so they run in parallel.
            nc.sync.dma_start(pm[:], din["pm"][:])
            nc.sync.dma_start(yT[:], din["yT"][:])
            nc.scalar.dma_start(xs[:], din["x16"][:])
            for t, n in ((w_trT5, "trT5"), (w_trT3, "trT3")):
                nc.scalar.dma_start(t[:], din[n][:])
            for t, n in ((w_GT5, "GT5"), (w_GT3, "GT3"), (biases, "biases")):
                nc.gpsimd.dma_start(t[:], din[n][:])
            for t, n in ((w_gk[0], "gkT1"), (w_gk[1], "gkT3"), (w_gk[2], "gkT5"),
                         (w_tr1, "tr1"), (w_G1T, "G1T"), (w_foxT, "foxT"),
                         (b1, "b1")):
                nc.sync.dma_start(t[:], din[n][:])

            # pooling on TensorE: pooled[c, j] = sum_px yT[px, c] * pm[px, j]
            # (pm carries the 1/count mean normalization and the zero row pad)
            for m in range(KC):
                psp = psum.tile([P, HALF], mybir.dt.float32, tag="ps",
                                name="psp")
                for ch in range(YCH):
                    nc.tensor.matmul(
                        psp[:, :35],
                        yT[:, ch, m * P:(m + 1) * P],
                        pm[:, ch, :],
                        start=(ch == 0), stop=(ch == YCH - 1))
                nc.vector.tensor_copy(pooled[:, m, :], psp[:, :35])

            # kernel-generator matmuls: kern = gk_w @ pooled + gk_b
            for w_g, off, kk2, gb in ((w_gk[2], OFF5, 25, GKB5),
                                      (w_gk[1], OFF3, 9, GKB3),
                                      (w_gk[0], OFF1, 1, GKB1)):
                for m in range(KC):
                    ps = psum.tile([P, HALF], mybir.dt.float32, tag="ps")
                    for kc in range(KC):
                        nc.tensor.matmul(
                            ps[:, :kk2],
                            w_g[:, kc, m * P:(m + 1) * P],
                            pooled[:, kc, off:off + kk2],
                            start=(kc == 0), stop=(kc == KC - 1))
                    nc.scalar.activation(
                        kern[:, m, off:off + kk2], ps[:, :kk2],
                        AF.Identity, bias=biases[:, gb + m:gb + m + 1])

            # k1 branch folded matrix: C1T = foxT + (tr1^T @ (G1T*kern1))
            for kc in range(KC):
                nc.vector.tensor_scalar_mul(
                    w_Gs1T[:, kc, :], w_G1T[:, kc, :], kern[:, kc, OFF1:OFF1 + 1])
            for mi in range(KC):
                ps = psum.tile([P, HALF], mybir.dt.float32, tag="ps")
                for kc in range(KC):
                    nc.tensor.matmul(
                        ps[:, :outC], w_tr1[:, kc, mi * P:(mi + 1) * P],
                        w_Gs1T[:, kc, :],
                        start=(kc == 0), stop=(kc == KC - 1))
                nc.vector.tensor_tensor(
                    w_C1T[:, mi, :], ps[:, :outC], w_foxT[:, mi, :], op=Alu.add)
            # out bias = b' + G1 @ (kern1 * b1)   (v1 via tiny matvec)
            for m in range(KC):
                ps = psum.tile([P, HALF], mybir.dt.float32, tag="ps")
                for kc in range(KC):
                    nc.tensor.matmul(
                        ps[:, :1], w_Gs1T[:, kc, m * P:(m + 1) * P],
                        b1[:, kc, :],
                        start=(kc == 0), stop=(kc == KC - 1))
                nc.vector.tensor_tensor(
                    biases[:, OB + m:OB + m + 1], ps[:, :1],
                    biases[:, BFO + m:BFO + m + 1], op=Alu.add)

        # ---------- stage 2+3: branches (tr + depthwise) and fused fo ----------
        # Depthwise split: PE taps run on TensorE as diag(kern) matmuls
        # accumulating in PSUM (the eviction initializes the accumulator);
        # the rest run on VectorE as tensor_scalar(4x) + tensor_tensor(2x).
        eye = wmain.tile([P, P], fp16, tag="eye")
        nc.sync.dma_start(eye[:], din["eye"][:])
        n_pe_max = max(len(PE_TAPS5), len(PE_TAPS3))

        # C1 @ x early (depends only on pooling chain, not on dw):
        # partial = C1.T@x + out_bias, stored fp16; the final fo eviction
        # adds it on VectorE (idle in the tail).
        fop = ctx.enter_context(tc.tile_pool(name="fop", bufs=1))
        partial = fop.tile([P, KC, HW], fp16, tag="partial")
        for m in range(KC):
            for half in range(2):
                ps = psum.tile([P, HALF], mybir.dt.float32, tag="ps",
                               name="psc1")
                for kc in range(KC):
                    for nb in range(HALF // NBLK):
                        nc.tensor.matmul(
                            ps[:, nb * NBLK:(nb + 1) * NBLK],
                            w_C1T[:, kc, m * P:(m + 1) * P],
                            xs[:, kc, half * HALF + nb * NBLK:
                               half * HALF + (nb + 1) * NBLK],
                            start=(kc == 0), stop=(kc == KC - 1))
                nc.scalar.activation(
                    partial[:, m, half * HALF:(half + 1) * HALF], ps[:],
                    AF.Identity, bias=biases[:, OB + m:OB + m + 1])

        with tc.tile_pool(name="pads", bufs=4) as pads, \
             tc.tile_pool(name="dtmp", bufs=1) as dtmp, \
             tc.tile_pool(name="diagp", bufs=2) as diagp, \
             tc.tile_pool(name="outp", bufs=2) as outp:
            br5 = (w_trT5, acc5, OFF5, 5, TRB5, PE_TAPS5, PE_GROUPS5)
            br3 = (w_trT3, acc3, OFF3, 3, TRB3, PE_TAPS3, PE_GROUPS3)
            units = []
            for m in range(KC):
                units += [(br5, m), (br3, m)]
            for br, m in units:
                if br == "fo5":
                    # acc5 complete: fold G5 @ acc5 into the fp16 partial now
                    # (fills the PE while the k3 units' DVE taps run)
                    for mo in range(KC):
                        for half in range(2):
                            ps = psum.tile([P, HALF], mybir.dt.float32,
                                           tag="ps", name="ps5")
                            for kc in range(KC):
                                for nb in range(HALF // NBLK):
                                    nc.tensor.matmul(
                                        ps[:, nb * NBLK:(nb + 1) * NBLK],
                                        w_GT5[:, kc, mo * P:(mo + 1) * P],
                                        acc5[:, kc, half * HALF + nb * NBLK:
                                             half * HALF + (nb + 1) * NBLK],
                                        start=(kc == 0), stop=(kc == KC - 1))
                            pslice = partial[:, mo, half * HALF:(half + 1) * HALF]
                            nc.vector.tensor_tensor(
                                pslice, ps[:], pslice, op=Alu.add)
                    continue
                w_tr, acc, koff, kk, trb, pe_taps, pe_groups = br
                p = kk // 2
                dve_taps = [(dy, dx) for dy in range(-p, p + 1)
                            for dx in range(-p, p + 1)
                            if (dy, dx) not in pe_taps]
                # build this unit's diag(kern) matrices (one op per group)
                diags = diagp.tile([P, n_pe_max, P], fp16, tag="diags")
                di = 0
                diag_idx = {}
                for start, step, count in pe_groups:
                    kv = kern[:, m, koff + start: koff + start + step * count]
                    kv = kv.rearrange("p (a b) -> p a b", b=step)[:, :, 0:1]
                    nc.vector.tensor_tensor(
                        diags[:, di:di + count, :],
                        eye[:].rearrange("p (o a) -> p o a", o=1).to_broadcast(
                            [P, count, P]),
                        kv.to_broadcast([P, count, P]),
                        op=Alu.mult)
                    for a in range(count):
                        t = start + a * step
                        diag_idx[(koff, m, t // kk - p, t % kk - p)] = di + a
                    di += count
                xpad = pads.tile([P, PR, PCE], fp16, tag="xpad")
                # zero the halo borders (interior is fully overwritten)
                nc.gpsimd.memset(xpad[:, 0:PAD, :], 0.0)
                nc.gpsimd.memset(xpad[:, PAD + H:PR, :], 0.0)
                nc.gpsimd.memset(xpad[:, PAD:PAD + H, 0:PAD], 0.0)
                nc.gpsimd.memset(xpad[:, PAD:PAD + H, PCE - PAD:PCE], 0.0)
                av = acc[:, m].rearrange("p (h w) -> p h w", w=W)
                for half in range(2):
                    ps = psum.tile([P, HALF], mybir.dt.float32, tag="ps")
                    for kc in range(KC):
                        for nb in range(HALF // NBLK):
                            nc.tensor.matmul(
                                ps[:, nb * NBLK:(nb + 1) * NBLK],
                                w_tr[:, kc, m * P:(m + 1) * P],
                                xs[:, kc, half * HALF + nb * NBLK:
                                   half * HALF + (nb + 1) * NBLK],
                                start=(kc == 0), stop=(kc == KC - 1))
                    psv = ps[:].rearrange("p (r c) -> p r c", c=W)
                    r0 = PAD + half * (H // 2)
                    nc.scalar.activation(
                        xpad[:, r0:r0 + H // 2, PAD:PAD + W], psv,
                        AF.Identity, bias=biases[:, trb + m:trb + m + 1])
                # PE taps: diag matmuls accumulate in PSUM, evict = init
                for half in range(2):
                    psd = psum.tile([P, HALF], mybir.dt.float32, tag="ps",
                                    name="psd")
                    for ti, (dy, dx) in enumerate(pe_taps):
                        dg = diags[:, diag_idx[(koff, m, dy, dx)], :]
                        for nb in range(4):
                            r0 = half * 32 + nb * 8
                            nc.tensor.matmul(
                                psd[:, nb * NBLK:(nb + 1) * NBLK],
                                dg,
                                xpad[:, PAD + dy + r0:PAD + dy + r0 + 8,
                                     PAD + dx:PAD + dx + W],
                                start=(ti == 0), stop=(ti == len(pe_taps) - 1))
                    nc.scalar.copy(
                        av[:, half * 32:half * 32 + 32, :],
                        psd[:].rearrange("p (r c) -> p r c", c=W))
                # DVE taps: mul at 4x into tmp, add at 2x into acc
                for (dy, dx) in dve_taps:
                    tap = koff + (dy + p) * kk + (dx + p)
                    tmp = dtmp.tile([P, HW], fp16, tag="dvetmp")
                    tv = tmp[:].rearrange("p (h w) -> p h w", w=W)
                    nc.vector.tensor_scalar_mul(
                        tv, xpad[:, PAD + dy:PAD + dy + H,
                                 PAD + dx:PAD + dx + W],
                        kern[:, m, tap:tap + 1])
                    nc.vector.tensor_tensor(av, tv, av, op=Alu.add)

            # ---------- tail: fo over the dw outputs + partial add ----------
            mats_fo = ((w_GT5, acc5), (w_GT3, acc3))
            for m in range(KC):
                for half in range(2):
                    ps = psum.tile([P, HALF], mybir.dt.float32, tag="ps",
                                   name="psfo")
                    for mi, (wm, rhs) in enumerate(mats_fo):
                        for kc in range(KC):
                            for nb in range(HALF // NBLK):
                                nc.tensor.matmul(
                                    ps[:, nb * NBLK:(nb + 1) * NBLK],
                                    wm[:, kc, m * P:(m + 1) * P],
                                    rhs[:, kc, half * HALF + nb * NBLK:
                                        half * HALF + (nb + 1) * NBLK],
                                    start=(mi == 0 and kc == 0),
                                    stop=(mi == len(mats_fo) - 1
                                          and kc == KC - 1))
                    for q in range(2):
                        o0 = half * HALF + q * (HALF // 2)
                        outs = outp.tile([P, HALF // 2], fp32, tag="outs")
                        nc.vector.tensor_tensor(
                            outs[:], ps[:, q * (HALF // 2):(q + 1) * (HALF // 2)],
                            partial[:, m, o0:o0 + HALF // 2],
                            op=Alu.add)
                        nc.sync.dma_start(
                            outd[:, m, o0:o0 + HALF // 2], outs[:])


def _chunk_pm(a):
    """[512, F...] -> [128, 4, F...] partition-major chunks."""
    return np.ascontiguousarray(
        a.reshape(KC, P, *a.shape[1:]).transpose(1, 0, *range(2, a.ndim + 1)))


def _prep_host(inputs):
    """Host-side weight folding + layout. Returns (shared weight arrays,
    per-core input arrays)."""
    f32 = np.float32
    x = np.asarray(inputs["x"], f32)
    y = np.asarray(inputs["y"], f32)
    gk_w = np.asarray(inputs["gk_w"], f32)
    gk_b = np.asarray(inputs["gk_b"], f32)
    tr_w = np.asarray(inputs["tr_w"], f32)
    tr_b = np.asarray(inputs["tr_b"], f32)
    fi_w = np.asarray(inputs["fi_w"], f32)
    fi_b = np.asarray(inputs["fi_b"], f32)
    fo_w = np.asarray(inputs["fo_w"], f32)
    fo_b = np.asarray(inputs["fo_b"], f32)

    fo_x = fo_w[:, :inC]
    fo_blk = [fo_w[:, inC + i * midC: inC + (i + 1) * midC] for i in range(3)]
    G = [fo_blk[i] @ fi_w[i] for i in range(3)]            # [outC, midC]
    bfo = fo_b + sum(fo_blk[i] @ fi_b[i] for i in range(3))

    f16 = np.float16
    shared = {
        "trT5": _chunk_pm(np.ascontiguousarray(tr_w[2].T).astype(f16)),
        "trT3": _chunk_pm(np.ascontiguousarray(tr_w[1].T).astype(f16)),
        "GT5": _chunk_pm(np.ascontiguousarray(G[2].T).astype(f16)),
        "GT3": _chunk_pm(np.ascontiguousarray(G[1].T).astype(f16)),
        "gkT1": _chunk_pm(np.ascontiguousarray(gk_w[0].T).astype(f16)),
        "gkT3": _chunk_pm(np.ascontiguousarray(gk_w[1].T).astype(f16)),
        "gkT5": _chunk_pm(np.ascontiguousarray(gk_w[2].T).astype(f16)),
        "tr1": _chunk_pm(tr_w[0].astype(f16)),
        "G1T": _chunk_pm(np.ascontiguousarray(G[0].T).astype(f16)),
        "foxT": _chunk_pm(np.ascontiguousarray(fo_x.T).astype(f16)),
        "b1": _chunk_pm(tr_b[0].astype(f16)[:, None]),
        "eye": np.eye(P, dtype=f16),
    }
    # pooling matrix [3600-padded, 35]: col j sums pixels of block j, scaled
    # by 1/count so the matmul produces the block means directly
    pmat = np.zeros((YCH * P, 35), np.float32)
    hw_idx = np.arange(Hy * Hy)
    hh, ww = hw_idx // Hy, hw_idx % Hy
    for j in range(25):
        jh, jw = j // 5, j % 5
        pmat[:Hy * Hy, OFF5 + j] = ((hh // (Hy // 5) == jh) &
                                    (ww // (Hy // 5) == jw)) / CNT5
    for j in range(9):
        jh, jw = j // 3, j % 3
        pmat[:Hy * Hy, OFF3 + j] = ((hh // (Hy // 3) == jh) &
                                    (ww // (Hy // 3) == jw)) / CNT3
    pmat[:Hy * Hy, OFF1] = 1.0 / CNT1
    shared["pm"] = np.ascontiguousarray(
        pmat.reshape(YCH, P, 35).transpose(1, 0, 2)).astype(f16)
    biases = np.zeros((P, 28), f32)
    for col, vec in ((TRB5, tr_b[2]), (TRB3, tr_b[1]), (GKB1, gk_b[0]),
                     (GKB3, gk_b[1]), (GKB5, gk_b[2]), (BFO, bfo)):
        biases[:, col:col + KC] = vec.reshape(KC, P).T
    shared["biases"] = biases

    per_core = []
    for b in range(B):
        yt = np.zeros((YCH * P, inC), f16)
        yt[:Hy * Hy] = y[b].reshape(inC, Hy * Hy).T.astype(f16)
        per_core.append({
            "x16": _chunk_pm(x[b].reshape(inC, HW).astype(f16)),
            "yT": np.ascontiguousarray(
                yt.reshape(YCH, P, inC).transpose(1, 0, 2)),
        })
    return shared, per_core


LAST_RESULTS = None


def _ensure_ntff_hook():
    """Best-effort: recreate the missing antenv.axon_hooks module so
    run_bass_kernel_spmd(trace=True) can capture NTFF profiles under axon."""
    import sys
    import types
    try:
        from antenv.axon_hooks import get_axon_ntff_profile_hook  # noqa: F401
        return
    except ImportError:
        pass
    try:
        import antenv
        from trn_agent_boot.trn_boot import _ntff_profile_via_ctypes
        mod = types.ModuleType("antenv.axon_hooks")
        mod._hook = None

        def set_axon_ntff_profile_hook(h):
            mod._hook = h

        def get_axon_ntff_profile_hook():
            return mod._hook

        mod.set_axon_ntff_profile_hook = set_axon_ntff_profile_hook
        mod.get_axon_ntff_profile_hook = get_axon_ntff_profile_hook
        sys.modules["antenv.axon_hooks"] = mod
        antenv.axon_hooks = mod
        mod.set_axon_ntff_profile_hook(
            _ntff_profile_via_ctypes("/opt/axon/libaxon_pjrt.so"))
    except Exception as e:  # profiling is optional — never break the run
        print(f"ntff hook unavailable: {e}")


def kernel(**inputs) -> np.ndarray:
    global LAST_RESULTS
    if "nc" not in _CACHED:
        _CACHED["nc"] = _build_program()
    nc = _CACHED["nc"]

    shared, per_core = _prep_host(inputs)
    in_maps = [{**shared, **pc} for pc in per_core]

    from concourse import bass_utils
    trace = bool(os.environ.get("DCM_TRACE"))
    if trace:
        _ensure_ntff_hook()
    res = bass_utils.run_bass_kernel_spmd(
        nc, in_maps, core_ids=list(range(N_CORES)), trace=trace)
    LAST_RESULTS = res

    out = np.empty((B, outC, H, W), np.float32)
    for b in range(B):
        o = res.results[b]["out"]                      # [128, KC, HW]
        out[b] = o.transpose(1, 0, 2).reshape(outC, H, W)
    return out
